# revision 1
# baseline (speedup 1.0000x reference)
"""3D Haar DWT (depth-1) Trainium2 kernel.

Full inputs: x [4, 4, 64, 256, 256] f32 + six banded Haar matrices
(hardcoded math: every output element is +-2^-1.5 times a +-sum of a
2x2x2 block). Returns the 8 subbands (LLL, LLH, LHL, LHH, HLL, HLH,
HHL, HHH), each [4, 4, 32, 128, 128] f32.

Sharding: data-parallel over N*C = 16 sample-channels, 2 per core on
8 cores. Per-core compute is a 3-stage butterfly over pair-packed
tiles (SBUF partition p holds input rows 2p and 2p+1 contiguously, so
every DMA descriptor is a 2 KiB linear run):
  H stage: row pairs    -> TensorE matmuls against +-2^-1.5 * I for
                           3 of every 4 d-pairs (fp32, exact);
                           DVE adds + ScalarE pre-scale for the 4th.
  W stage: column pairs -> DVE stride-2 tensor_add/sub (FD=1024)
  D stage: slice pairs  -> DVE tensor_add/sub (FD=1024, 4-D APs
                           covering two subbands per instruction)
ScalarE evacuates PSUM. Everything stays fp32-exact.
"""
import sys

sys.path.insert(0, "/opt/trn_rl_repo")

import numpy as np

N, C, D, H, W = 4, 4, 64, 256, 256
NCORES = 8
G_PER_CORE = (N * C) // NCORES        # 2
KP = D // 2                           # 32 d-pairs per g
S3 = float(2.0 ** -1.5)

# schedule tunables
KB = 8                                # k-slices per output staging block
IN_BUFS = 8
EV_BUFS = 4
WT_BUFS = 2
OS_BUFS = 2
PSUM_BUFS = 3

_CACHE = {}


def _build_filter_lhst():
    """Stationary operands: +S3*I and -S3*I, as [2, 128, 128] fp32."""
    eye = np.eye(128, dtype=np.float32)
    return np.stack([np.float32(S3) * eye, np.float32(-S3) * eye])


def _build_nc():
    import concourse.bass as bass
    import concourse.tile as tile
    from concourse import bacc, mybir

    f32 = mybir.dt.float32
    nc = bacc.Bacc(None)
    x_d = nc.declare_dram_parameter("x", [G_PER_CORE, D, H, W], f32,
                                    isOutput=False)
    ft_d = nc.declare_dram_parameter("ft", [2, 128, 128], f32,
                                     isOutput=False)
    # h'-major layout: per (s, g, partition=h') a k-block of 8 is one
    # contiguous 4 KiB run in DRAM (host transposes k and h' back)
    o_d = nc.declare_dram_parameter("out", [8, G_PER_CORE, 128, KP, 128],
                                    f32, isOutput=True)

    with tile.TileContext(nc) as tc:
        with (
            tc.tile_pool(name="cst", bufs=1) as cst,
            tc.tile_pool(name="inp", bufs=IN_BUFS) as inp,
            tc.tile_pool(name="ev", bufs=EV_BUFS) as evp,
            tc.tile_pool(name="wt", bufs=WT_BUFS) as wtp,
            tc.tile_pool(name="os", bufs=OS_BUFS) as osp,
            tc.tile_pool(name="ps", bufs=PSUM_BUFS, space="PSUM") as psp,
        ):
            ft = cst.tile([128, 256], f32, tag="ft")
            nc.sync.dma_start(
                ft.rearrange("p (i c) -> p i c", i=2),
                ft_d.rearrange("i p c -> p i c"))
            pos_i = ft[:, 0:128]    # +S3 * I
            neg_i = ft[:, 128:256]  # -S3 * I

            def load_pair(g, k):
                """One d-pair as a pair-packed tile [128, 1024]:
                cols = {s0: row2p row2p+1 | s1: row2p row2p+1}."""
                t = inp.tile([128, 1024], f32, tag="xin")
                nc.sync.dma_start(
                    t.rearrange("p (s r) -> p s r", s=2),
                    x_d[g, 2 * k:2 * k + 2].rearrange(
                        "s (p r) w -> p s (r w)", r=2))
                return t

            for g in range(G_PER_CORE):
                for kb in range(KP // KB):
                    os_t = osp.tile([128, 8 * KB * 128], f32, tag="os")
                    for half in range(KB // 4):
                        wt_t = wtp.tile([128, 4 * 1024], f32, tag="wt")
                        # EV tiles: j0+j1 (both PE), j2 (PE) + j3 (DVE)
                        ev01 = evp.tile([128, 2048], f32, tag="ev")
                        ev23 = evp.tile([128, 2048], f32, tag="ev")
                        for j in range(4):
                            k = kb * KB + half * 4 + j
                            t = load_pair(g, k)
                            t4 = t.rearrange("p (s r w) -> p s r w",
                                             s=2, r=2)
                            if j < 3:
                                # --- H stage on TensorE: +-S3*I matmuls
                                pt = psp.tile([128, 1024], f32, tag="ps")
                                lo = pt[:, 0:512].rearrange(
                                    "p (s w) -> p s w", s=2)
                                hi = pt[:, 512:1024].rearrange(
                                    "p (s w) -> p s w", s=2)
                                nc.tensor.matmul(lo, pos_i, t4[:, :, 0, :],
                                                 start=True, stop=False)
                                nc.tensor.matmul(lo, pos_i, t4[:, :, 1, :],
                                                 start=False, stop=True)
                                nc.tensor.matmul(hi, pos_i, t4[:, :, 0, :],
                                                 start=True, stop=False)
                                nc.tensor.matmul(hi, neg_i, t4[:, :, 1, :],
                                                 start=False, stop=True)
                                # ScalarE evacuation (scaled via weights)
                                dst = (ev01[:, j * 1024:(j + 1) * 1024]
                                       if j < 2 else ev23[:, 0:1024])
                                nc.scalar.activation(
                                    dst, pt[:],
                                    mybir.ActivationFunctionType.Copy)
                            else:
                                # --- H stage on DVE (ScalarE pre-scale)
                                nc.scalar.activation(
                                    t[:], t[:],
                                    mybir.ActivationFunctionType.Copy,
                                    bias=0.0, scale=S3)
                                pl = ev23[:, 1024:2048]
                                pl3 = pl.rearrange("p (b s w) -> p b s w",
                                                   b=2, s=2)
                                nc.vector.tensor_add(
                                    pl3[:, 0], t4[:, :, 0, :],
                                    t4[:, :, 1, :])
                                nc.vector.tensor_sub(
                                    pl3[:, 1], t4[:, :, 0, :],
                                    t4[:, :, 1, :])
                        # --- W stage on DVE, FD=1024 ---
                        # ev layout per 1024: {A_lo(s0,s1) | A_hi(s0,s1)}
                        wt4 = wt_t.rearrange("p (j b) -> p j b", j=4)
                        for ev, j0 in ((ev01, 0), (ev23, 2)):
                            nc.vector.tensor_add(
                                wt4[:, j0:j0 + 2, 0:512],
                                ev[:, 0::2].rearrange(
                                    "p (j b) -> p j b", j=2),
                                ev[:, 1::2].rearrange(
                                    "p (j b) -> p j b", j=2))
                            nc.vector.tensor_sub(
                                wt4[:, j0:j0 + 2, 512:1024],
                                ev[:, 0::2].rearrange(
                                    "p (j b) -> p j b", j=2),
                                ev[:, 1::2].rearrange(
                                    "p (j b) -> p j b", j=2))
                        # --- D stage, FD=1024, two subbands per op ---
                        # wt_t per-pair block (j): {LL0 LL1 HL0 HL1 |
                        #                           LH0 LH1 HH0 HH1}
                        wtd = wt_t.rearrange("p (j c w) -> p c j w",
                                             j=4, c=8)
                        osd = os_t.rearrange("p (s q w) -> p s q w",
                                             s=8, q=KB)
                        qs = slice(half * 4, half * 4 + 4)
                        for c0, s_sum, s_diff in ((0, 0, 4), (4, 1, 5)):
                            # c blocks {c0, c0+2} = {LL,HL} / {LH,HH}
                            in0 = wtd[:, c0:c0 + 3:2]
                            in1 = wtd[:, c0 + 1:c0 + 4:2]
                            nc.vector.tensor_add(
                                osd[:, s_sum:s_sum + 3:2, qs], in0, in1)
                            nc.vector.tensor_sub(
                                osd[:, s_diff:s_diff + 3:2, qs], in0, in1)
                    # --- store this k-block: 8 subbands x [128,KB,128] ---
                    for s in range(8):
                        src_ap = os_t[:, s * KB * 128:(s + 1) * KB * 128]
                        nc.sync.dma_start(
                            o_d[s, g, :, kb * KB:(kb + 1) * KB, :],
                            src_ap.rearrange("p (q w) -> p q w", q=KB))
    nc.finalize()
    return nc


def _get_nc():
    if "nc" not in _CACHE:
        _CACHE["nc"] = _build_nc()
    return _CACHE["nc"]


def kernel(x, low_0, low_1, low_2, high_0, high_1, high_2):
    from concourse.bass_utils import run_bass_kernel_spmd

    x = np.ascontiguousarray(np.asarray(x, dtype=np.float32))
    ft = _build_filter_lhst()
    xs = x.reshape(N * C, D, H, W)
    in_maps = [
        {"x": np.ascontiguousarray(
            xs[c * G_PER_CORE:(c + 1) * G_PER_CORE]), "ft": ft}
        for c in range(NCORES)
    ]
    nc = _get_nc()
    res = run_bass_kernel_spmd(nc, in_maps, list(range(NCORES)))
    full = np.empty((8, N * C, KP, 128, 128), dtype=np.float32)
    for c in range(NCORES):
        full[:, c * G_PER_CORE:(c + 1) * G_PER_CORE] = \
            res.results[c]["out"].transpose(0, 1, 3, 2, 4)
    full = full.reshape(8, N, C, KP, 128, 128)
    return tuple(full[s] for s in range(8))



# revision 2
# speedup vs baseline: 1.9451x; 1.9451x over previous
"""3D Haar DWT (depth-1) Trainium2 kernel — bf16 pipeline.

Full inputs: x [4, 4, 64, 256, 256] f32 + six banded Haar matrices
(hardcoded math: every output element is +-2^-1.5 times a +-sum of a
2x2x2 block). Returns the 8 subbands (LLL..HHH), each
[4, 4, 32, 128, 128] f32.

Strategy: data-parallel over N*C = 16 sample-channels, 2 per core.
The 2e-2 tolerance admits bf16, which halves HBM traffic (the
roofline bottleneck) and doubles DVE throughput. The host pre-scales
x by 2^-1.5, casts to bf16, and pre-permutes so that every DMA is a
dense [128, F] block with 8 KiB per-partition runs.

Per-core device pipeline, per (g, kd-block-of-4):
  DMA in   [128, 4096] bf16   p = (dd, dh, q=h'%32), f = (kd, hi, dw, w')
  DVE      W stage: even/odd-w halves add/sub (bf16 2x mode, FD=2048)
  TensorE  H+D stages fused in ONE matmul per 512 cols against a
           stationary 128x128 +-1 butterfly matrix (4 nonzeros/col):
           out partition (sd, sh, q) = sum of (dd, dh, q) partitions
  ScalarE/ PSUM fp32 -> SBUF bf16 evacuation (3 of 4 tiles on
   VectorE  ScalarE Copy, 1 of 4 on DVE tensor_copy)
  DMA out  [128, 4096] bf16
Engine budgets/core: DMA ~94us (bound), DVE ~55us, ScalarE ~55us,
PE ~27us.
"""
import sys

sys.path.insert(0, "/opt/trn_rl_repo")

import numpy as np
import ml_dtypes

BF16 = ml_dtypes.bfloat16

N, C, D, H, W = 4, 4, 64, 256, 256
NCORES = 8
G_PER_CORE = (N * C) // NCORES        # 2
KD = D // 2                           # 32 d-pairs
KB = 4                                # kd per DMA block
NBLK = KD // KB                       # 8 blocks per g
S3 = np.float32(2.0 ** -1.5)

IN_BUFS = 3
WT_BUFS = 3
OUT_BUFS = 3
PSUM_BUFS = 3

_CACHE = {}


def _build_butterfly():
    """lhsT[p_in, p_out]: p_in = dd*64+dh*32+q, p_out = (sd*2+sh)*32+q,
    value (-1)^(dd*sd + dh*sh). Exact in bf16."""
    m = np.zeros((128, 128), dtype=np.float32)
    dd = np.arange(2)[:, None, None, None, None]
    dh = np.arange(2)[None, :, None, None, None]
    sd = np.arange(2)[None, None, :, None, None]
    sh = np.arange(2)[None, None, None, :, None]
    q = np.arange(32)[None, None, None, None, :]
    p_in = (dd * 64 + dh * 32 + q).astype(np.int64)
    p_out = ((sd * 2 + sh) * 32 + q).astype(np.int64)
    val = (-1.0) ** (dd * sd + dh * sh)
    bi = np.broadcast_arrays(p_in, p_out, val)
    m[bi[0].ravel(), bi[1].ravel()] = bi[2].ravel()
    return m.astype(BF16)


def _pack_inputs(x):
    """x [4,4,64,256,256] f32 -> xb [8 cores, 2, 128, 32768] bf16,
    pre-scaled by 2^-1.5. p=(dd,dh,q); f=(kd,hi,dw,w')."""
    xs = (np.asarray(x, np.float32).reshape(16, 64, 256, 256) * S3)
    xs = xs.astype(BF16)
    # c g kd dd hi q dh w' dw
    v = xs.reshape(8, 2, 32, 2, 4, 32, 2, 128, 2)
    # -> c g dd dh q kd hi dw w'
    v = v.transpose(0, 1, 3, 6, 5, 2, 4, 8, 7)
    return np.ascontiguousarray(v.reshape(8, 2, 128, KD * 1024))


def _unpack_outputs(ob_all):
    """ob_all [8 cores, 2, 128, 32768] bf16 -> tuple of 8 bands
    [4,4,32,128,128] f32. p'=(dh,q); f=(kd,sw,hi,w')."""
    v = np.asarray(ob_all).reshape(8, 2, 4, 32, 32, 2, 4, 128)
    # c g dh q kd sw hi w' -> dh sw c g kd hi q w'
    v = v.transpose(2, 5, 0, 1, 4, 6, 3, 7)
    out = np.ascontiguousarray(v).astype(np.float32)
    out = out.reshape(8, 4, 4, 32, 128, 128)
    return tuple(out[s] for s in range(8))


def _build_nc():
    import concourse.bass as bass
    import concourse.tile as tile
    from concourse import bacc, mybir

    f32 = mybir.dt.float32
    bf16 = mybir.dt.bfloat16
    nc = bacc.Bacc(None)
    xb_d = nc.declare_dram_parameter("xb", [G_PER_CORE, 128, KD * 1024],
                                     bf16, isOutput=False)
    wt_d = nc.declare_dram_parameter("wt", [128, 128], bf16,
                                     isOutput=False)
    ob_d = nc.declare_dram_parameter("ob", [G_PER_CORE, 128, KD * 1024],
                                     bf16, isOutput=True)
    copy_f = mybir.ActivationFunctionType.Copy

    with tile.TileContext(nc) as tc:
        with (
            tc.tile_pool(name="cst", bufs=1) as cst,
            tc.tile_pool(name="inp", bufs=IN_BUFS) as inp,
            tc.tile_pool(name="wst", bufs=WT_BUFS) as wst,
            tc.tile_pool(name="out", bufs=OUT_BUFS) as outp,
            tc.tile_pool(name="ps", bufs=PSUM_BUFS, space="PSUM") as psp,
        ):
            bt = cst.tile([128, 128], bf16, tag="bt")
            nc.sync.dma_start(bt[:, :], wt_d[:, :])

            for g in range(G_PER_CORE):
                for kb in range(NBLK):
                    sl = slice(kb * KB * 1024, (kb + 1) * KB * 1024)
                    tin = inp.tile([128, KB * 1024], bf16, tag="tin")
                    nc.sync.dma_start(tin[:, :], xb_d[g, :, sl])
                    tw = wst.tile([128, KB * 1024], bf16, tag="tw")
                    tout = outp.tile([128, KB * 1024], bf16, tag="tout")
                    # --- W stage on DVE (bf16 2x): even/odd-w halves
                    i5 = tin.rearrange("p (k hi dw w) -> p k hi dw w",
                                       k=KB, hi=4, dw=2)
                    w5 = tw.rearrange("p (k sw hi w) -> p k sw hi w",
                                      k=KB, sw=2, hi=4)
                    nc.vector.tensor_add(w5[:, :, 0], i5[:, :, :, 0, :],
                                         i5[:, :, :, 1, :])
                    nc.vector.tensor_sub(w5[:, :, 1], i5[:, :, :, 0, :],
                                         i5[:, :, :, 1, :])
                    # --- H+D stages fused on TensorE; evac per kd
                    for j in range(KB):
                        ps = psp.tile([128, 1024], f32, tag="ps")
                        base = j * 1024
                        nc.tensor.matmul(ps[:, 0:512], bt[:, :],
                                         tw[:, base:base + 512],
                                         start=True, stop=True)
                        nc.tensor.matmul(ps[:, 512:1024], bt[:, :],
                                         tw[:, base + 512:base + 1024],
                                         start=True, stop=True)
                        dst = tout[:, base:base + 1024]
                        if j == KB - 1:
                            nc.vector.tensor_copy(dst, ps[:, :])
                        else:
                            nc.scalar.activation(dst, ps[:, :], copy_f)
                    nc.sync.dma_start(ob_d[g, :, sl], tout[:, :])
    nc.finalize()
    return nc


def _get_nc():
    if "nc" not in _CACHE:
        _CACHE["nc"] = _build_nc()
    return _CACHE["nc"]


def _prepare_in_maps(x):
    xb = _pack_inputs(x)
    wt = _build_butterfly()
    return [{"xb": np.ascontiguousarray(xb[c]), "wt": wt}
            for c in range(NCORES)]


def kernel(x, low_0, low_1, low_2, high_0, high_1, high_2):
    from concourse.bass_utils import run_bass_kernel_spmd

    in_maps = _prepare_in_maps(x)
    nc = _get_nc()
    res = run_bass_kernel_spmd(nc, in_maps, list(range(NCORES)))
    ob_all = np.stack([np.asarray(res.results[c]["ob"])
                       for c in range(NCORES)])
    return _unpack_outputs(ob_all)


# revision 4
# speedup vs baseline: 2.2270x; 1.1449x over previous
"""3D Haar DWT (depth-1) Trainium2 kernel — bf16 pipeline.

Full inputs: x [4, 4, 64, 256, 256] f32 + six banded Haar matrices
(hardcoded math: every output element is +-2^-1.5 times a +-sum of a
2x2x2 block). Returns the 8 subbands (LLL..HHH), each
[4, 4, 32, 128, 128] f32.

Strategy: data-parallel over N*C = 16 sample-channels, 2 per core.
The 2e-2 tolerance admits bf16, which halves HBM traffic (the
roofline bottleneck) and doubles DVE throughput. The host pre-scales
x by 2^-1.5, casts to bf16, and pre-permutes so that every DMA is a
dense [128, F] block with 8 KiB per-partition runs.

Per-core device pipeline, per (g, kd-block-of-4):
  DMA in   [128, 4096] bf16   p = (dd, dh, q=h'%32), f = (kd, hi, dw, w')
  DVE      W stage: even/odd-w halves add/sub (bf16 2x mode, FD=2048)
  TensorE  H+D stages fused in ONE matmul per 512 cols against a
           stationary 128x128 +-1 butterfly matrix (4 nonzeros/col):
           out partition (sd, sh, q) = sum of (dd, dh, q) partitions
  ScalarE/ PSUM fp32 -> SBUF bf16 evacuation (3 of 4 tiles on
   VectorE  ScalarE Copy, 1 of 4 on DVE tensor_copy)
  DMA out  [128, 4096] bf16
Engine budgets/core: DMA ~94us (bound), DVE ~55us, ScalarE ~55us,
PE ~27us.
"""
import sys

sys.path.insert(0, "/opt/trn_rl_repo")

import numpy as np
import ml_dtypes

BF16 = ml_dtypes.bfloat16

N, C, D, H, W = 4, 4, 64, 256, 256
NCORES = 8
G_PER_CORE = (N * C) // NCORES        # 2
KD = D // 2                           # 32 d-pairs
KB = 2                                # kd per DMA block
NBLK = KD // KB                       # 16 blocks per g
S3 = np.float32(2.0 ** -1.5)

IN_BUFS = 4
WT_BUFS = 4
OUT_BUFS = 4
PSUM_BUFS = 4

_CACHE = {}


def _build_butterfly():
    """lhsT[p_in, p_out]: p_in = dd*64+dh*32+q, p_out = (sd*2+sh)*32+q,
    value (-1)^(dd*sd + dh*sh). Exact in bf16."""
    m = np.zeros((128, 128), dtype=np.float32)
    dd = np.arange(2)[:, None, None, None, None]
    dh = np.arange(2)[None, :, None, None, None]
    sd = np.arange(2)[None, None, :, None, None]
    sh = np.arange(2)[None, None, None, :, None]
    q = np.arange(32)[None, None, None, None, :]
    p_in = (dd * 64 + dh * 32 + q).astype(np.int64)
    p_out = ((sd * 2 + sh) * 32 + q).astype(np.int64)
    val = (-1.0) ** (dd * sd + dh * sh)
    bi = np.broadcast_arrays(p_in, p_out, val)
    m[bi[0].ravel(), bi[1].ravel()] = bi[2].ravel()
    return m.astype(BF16)


def _pack_inputs(x):
    """x [4,4,64,256,256] f32 -> xb [8 cores, 2, 128, 32768] bf16,
    pre-scaled by 2^-1.5. p=(dd,dh,q); f=(kd,hi,dw,w')."""
    xs = (np.asarray(x, np.float32).reshape(16, 64, 256, 256) * S3)
    xs = xs.astype(BF16)
    # c g kd dd hi q dh w' dw
    v = xs.reshape(8, 2, 32, 2, 4, 32, 2, 128, 2)
    # -> c g dd dh q kd hi dw w'
    v = v.transpose(0, 1, 3, 6, 5, 2, 4, 8, 7)
    return np.ascontiguousarray(v.reshape(8, 2, 128, KD * 1024))


def _unpack_outputs(ob_all):
    """ob_all [8 cores, 2, 128, 32768] bf16 -> tuple of 8 bands
    [4,4,32,128,128] f32. p'=(dh,q); f=(kd,sw,hi,w')."""
    v = np.asarray(ob_all).reshape(8, 2, 4, 32, 32, 2, 4, 128)
    # c g dh q kd sw hi w' -> dh sw c g kd hi q w'
    v = v.transpose(2, 5, 0, 1, 4, 6, 3, 7)
    out = np.ascontiguousarray(v).astype(np.float32)
    out = out.reshape(8, 4, 4, 32, 128, 128)
    return tuple(out[s] for s in range(8))


def _build_nc():
    import concourse.bass as bass
    import concourse.tile as tile
    from concourse import bacc, mybir

    f32 = mybir.dt.float32
    bf16 = mybir.dt.bfloat16
    nc = bacc.Bacc(None)
    xb_d = nc.declare_dram_parameter("xb", [G_PER_CORE, 128, KD * 1024],
                                     bf16, isOutput=False)
    wt_d = nc.declare_dram_parameter("wt", [128, 128], bf16,
                                     isOutput=False)
    ob_d = nc.declare_dram_parameter("ob", [G_PER_CORE, 128, KD * 1024],
                                     bf16, isOutput=True)
    copy_f = mybir.ActivationFunctionType.Copy

    with tile.TileContext(nc) as tc:
        with (
            tc.tile_pool(name="cst", bufs=1) as cst,
            tc.tile_pool(name="inp", bufs=IN_BUFS) as inp,
            tc.tile_pool(name="wst", bufs=WT_BUFS) as wst,
            tc.tile_pool(name="out", bufs=OUT_BUFS) as outp,
            tc.tile_pool(name="ps", bufs=PSUM_BUFS, space="PSUM") as psp,
        ):
            bt = cst.tile([128, 128], bf16, tag="bt")
            nc.sync.dma_start(bt[:, :], wt_d[:, :])

            for g in range(G_PER_CORE):
                for kb in range(NBLK):
                    sl = slice(kb * KB * 1024, (kb + 1) * KB * 1024)
                    tin = inp.tile([128, KB * 1024], bf16, tag="tin")
                    nc.sync.dma_start(tin[:, :], xb_d[g, :, sl])
                    tw = wst.tile([128, KB * 1024], bf16, tag="tw")
                    tout = outp.tile([128, KB * 1024], bf16, tag="tout")
                    # --- W stage on DVE (bf16 2x): even/odd-w halves
                    i5 = tin.rearrange("p (k hi dw w) -> p k hi dw w",
                                       k=KB, hi=4, dw=2)
                    w5 = tw.rearrange("p (k sw hi w) -> p k sw hi w",
                                      k=KB, sw=2, hi=4)
                    nc.vector.tensor_add(w5[:, :, 0], i5[:, :, :, 0, :],
                                         i5[:, :, :, 1, :])
                    nc.vector.tensor_sub(w5[:, :, 1], i5[:, :, :, 0, :],
                                         i5[:, :, :, 1, :])
                    # --- H+D stages fused on TensorE; evac per kd.
                    # 3-of-4 evacs on ScalarE, 1-of-4 on DVE; the last
                    # block of the pipeline drains with one of each in
                    # parallel.
                    for j in range(KB):
                        ps = psp.tile([128, 1024], f32, tag="ps")
                        base = j * 1024
                        nc.tensor.matmul(ps[:, 0:512], bt[:, :],
                                         tw[:, base:base + 512],
                                         start=True, stop=True)
                        nc.tensor.matmul(ps[:, 512:1024], bt[:, :],
                                         tw[:, base + 512:base + 1024],
                                         start=True, stop=True)
                        dst = tout[:, base:base + 1024]
                        if j == KB - 1 and kb % 2 == 1:
                            nc.vector.tensor_copy(dst, ps[:, :])
                        else:
                            nc.scalar.activation(dst, ps[:, :], copy_f)
                    # stores issue from the second HWDGE engine (ACT) to
                    # keep the sync NX queue short
                    nc.scalar.dma_start(ob_d[g, :, sl], tout[:, :])
    nc.finalize()
    return nc


def _get_nc():
    if "nc" not in _CACHE:
        _CACHE["nc"] = _build_nc()
    return _CACHE["nc"]


def _prepare_in_maps(x):
    xb = _pack_inputs(x)
    wt = _build_butterfly()
    return [{"xb": np.ascontiguousarray(xb[c]), "wt": wt}
            for c in range(NCORES)]


def kernel(x, low_0, low_1, low_2, high_0, high_1, high_2):
    from concourse.bass_utils import run_bass_kernel_spmd

    in_maps = _prepare_in_maps(x)
    nc = _get_nc()
    res = run_bass_kernel_spmd(nc, in_maps, list(range(NCORES)))
    ob_all = np.stack([np.asarray(res.results[c]["ob"])
                       for c in range(NCORES)])
    return _unpack_outputs(ob_all)
